# revision 74
# baseline (speedup 1.0000x reference)
"""LSTM regression kernel for 8 Trainium2 NeuronCores.

Model (reference): B=2048, IN=2048, H=1024, T=15 steps, x constant across
steps. Data-parallel over batch: each of the 8 cores handles 256 batch rows.

Device strategy (per core, batch BL=256):
 - Everything kept "transposed": state hT/cT stored as [H, BL] with H on
   partitions (8 chunks of 128), so no per-step transposes are needed.
 - Phase A (fused xg + step 0): xgT[4H, BL] = W_ih @ xT in a single fp16
   pass. The gate bias b_ih+b_hh rides along as the per-partition scalar
   operand of the DVE op that saves xg to SBUF. Since h0 is the constant
   0.01, step 0's recurrent term is the per-row constant 0.01*rowsum(W_hh),
   folded into the step-0 gate activations as the per-partition ACT bias --
   step 0 needs NO recurrent matmuls.
 - Phase B (steps 1..14): gatesT[4H, BL] = W_hh @ hT accumulated in PSUM
   over 8 K-chunks; the precomputed xg tile is added in-place into PSUM on
   the DVE (cheaper than identity-weight matmuls on the busy PE).
 - Activations (sigmoid/tanh) on ScalarE directly from PSUM; cell update on
   VectorE per 128-row h-chunk so it pipelines with the matmuls.
 - Matmul inputs in fp16 (fp32 PSUM accumulate); h kept fp16; hidden-state
   outputs stored fp16 and upconverted host-side.
 - DMA choreography matters as much as the engines: the DMA unit moves
   ~0.36 GB/ms serially, so weight tiles are sized/ordered so delivery just
   leads consumption (sweep-0 singles first, W_hh streamed behind the wih
   sweeps, consumed via a kc7-deferral right after the phase switch).
"""

import os
import numpy as np

try:
    import concourse.bass as bass
except ImportError:  # pragma: no cover
    import sys
    sys.path.insert(0, "/opt/trn_rl_repo")
    import concourse.bass as bass
from concourse import bacc
import concourse.mybir as mybir
import concourse.tile as tile
from concourse.bass_utils import run_bass_kernel_spmd

F32 = mybir.dt.float32
F16 = mybir.dt.float16
AF = mybir.ActivationFunctionType
ALU = mybir.AluOpType

T = 15
B, IN, H = 2048, 2048, 1024
NCORES = 8
BL = B // NCORES            # 256 batch rows per core
G4 = 4 * H                  # 4096 gate rows
NM = G4 // 128              # 32 gate m-tiles
NKH = H // 128              # 8 hidden K-chunks
NKX = IN // 128             # 16 input K-chunks (biases folded via ACT bias)
NQ = 4                      # wih quad-chunks per sweep (4 kc each)
NSW = 4                     # sweeps; sweep s covers hc pair (2s, 2s+1)
INIT = 0.01

LAST_EXEC_NS = None
LAST_RESULTS = None

_cached_nc = None


def _build():
    nc = bacc.Bacc(None, target_bir_lowering=False)
    wih = nc.dram_tensor("wih", [NSW, NQ, 128, 4 * 1024], F16, kind="ExternalInput")
    whh = nc.dram_tensor("whh", [NKH, 128, G4], F16, kind="ExternalInput")
    xt = nc.dram_tensor("xt", [NKX, 128, BL], F16, kind="ExternalInput")
    rb = nc.dram_tensor("rb", [128, NM], F32, kind="ExternalInput")
    bb = nc.dram_tensor("bb", [128, NM], F32, kind="ExternalInput")
    hs = nc.dram_tensor("hs", [T, 128, NKH * BL], F16, kind="ExternalOutput")

    with tile.TileContext(nc) as tc:
        with (
            tc.tile_pool(name="const", bufs=1) as constp,
            tc.tile_pool(name="wihp", bufs=3) as wihp,
            tc.tile_pool(name="w0p", bufs=2) as w0p,
            tc.tile_pool(name="w1p", bufs=1) as w1p,
            tc.tile_pool(name="state", bufs=2) as statep,
            tc.tile_pool(name="gates", bufs=5) as gatesp,
            tc.tile_pool(name="psum", bufs=8, space="PSUM") as psump,
        ):
            xt_sbq = [
                constp.tile([128, 4 * BL], F16, tag=f"xtq{q}", name=f"xtq{q}")
                for q in range(4)
            ]
            r_sb = constp.tile([128, NM], F32, tag="rb")
            bb_sb = constp.tile([128, NM], F32, tag="bb")
            xg_sb = constp.tile([128, NM * BL], F16, tag="xg")
            # step-1 pass-1 gate strip (i/f gates for all 8 hc)
            g01_sb = constp.tile([128, 16 * BL], F16, tag="g01")
            # W_hh per k-chunk split into gate-halves (16 m-tiles each) so
            # phase B can start consuming a chunk ~1.5us before the full
            # 1MB chunk has landed
            whh_sb = [
                [
                    constp.tile(
                        [128, G4 // 2], F16,
                        tag=f"whh{kc}_{h}", name=f"whh{kc}_{h}"
                    )
                    for h in range(2)
                ]
                for kc in range(NKH)
            ]

            # PE warm-up: a tiny matmul at t~1us starts the p-state ramp
            # early, so sweep 0 runs at the full 2.4GHz clock (the ramp
            # window tracks wall time from the first PE instruction).
            warm = constp.tile([128, 8], F16, tag="warm")
            nc.vector.memset(warm[:, :], 0.0)
            warm_ps = psump.tile([128, 8], F32, tag="ps", name="warmps")
            nc.tensor.matmul(
                warm_ps[:8, :], warm[:, :], warm[:, :8],
                start=True, stop=True,
            )

            # Sweep 0's wih as 16 single-kc tiles interleaved with the x
            # quarters: the DMA unit transfers serially (~0.7us per 256KB),
            # so fine-grained tiles keep delivery just ahead of the PE's
            # ~0.93us/chunk consumption from the very first matmul.
            w0s = [
                w0p.tile([128, 1024], F16, tag=f"w0s{i % 4}", name=f"w0s_{i}")
                for i in range(NKX)
            ]

            def _dma_w0(i):
                nc.sync.dma_start(
                    w0s[i][:, :], wih[0, i // 4][:, (i % 4) * 1024:
                                                 (i % 4 + 1) * 1024]
                )

            def _dma_xtq(q):
                nc.sync.dma_start(
                    xt_sbq[q][:, :].rearrange("p (kc b) -> p kc b", kc=4),
                    xt[4 * q:4 * q + 4].rearrange("kc p b -> p kc b"),
                )

            _dma_w0(0)
            _dma_xtq(0)
            for i in (1, 2, 3):
                _dma_w0(i)
            _dma_xtq(1)
            for i in (4, 5, 6, 7):
                _dma_w0(i)
            _dma_xtq(2)
            nc.sync.dma_start(r_sb[:, :], rb[:, :])
            nc.sync.dma_start(bb_sb[:, :], bb[:, :])
            for i in (8, 9, 10, 11):
                _dma_w0(i)
            _dma_xtq(3)
            for i in (12, 13, 14, 15):
                _dma_w0(i)
            # Sweep 1's first quad also as singles, jumping ahead of r/bb:
            # quads land in one 2.9us lump, which is exactly the hiccup seen
            # at each sweep boundary. Singles smooth the handoff.
            w1s = [
                w0p.tile([128, 1024], F16, tag=f"w0s{i % 4}", name=f"w1s_{i}")
                for i in range(4)
            ]
            for i in range(4):
                nc.sync.dma_start(
                    w1s[i][:, :], wih[1, 0][:, i * 1024:(i + 1) * 1024]
                )

            # ---- Phase A: xg = W_ih @ x (single fp16 pass) fused with
            # step 0 (per-partition ACT bias carries b_ih+b_hh and the
            # constant-h0 recurrent term 0.01*rowsum(W_hh)).
            h0 = statep.tile([128, NKH * BL], F16, tag="h")
            c0 = statep.tile([128, NKH * BL], F32, tag="c")
            for s in range(NSW):
                # One [128,256] PSUM tile per (gate, hc-of-pair): a PSUM bank
                # admits only ONE pending accumulation group (2KB zero
                # region), so tiles must not share banks across groups.
                waves = [range(8)]
                sweep_w = {}

                def _lhs(qc, kci, ml):
                    key = (qc,)
                    if key not in sweep_w:
                        if s == 0:
                            sweep_w[key] = [w0s[qc * 4 + i] for i in range(4)]
                        elif s == 1 and qc == 0:
                            sweep_w[key] = w1s
                        elif s == 1:
                            # the faster warmed-up sweep 0 reaches these
                            # quads right at their arrival; singles keep
                            # each wait under the p-state reset threshold.
                            # q1 uses fresh 1-buf tags so its DMAs need no
                            # WAR slot release before starting.
                            pool = w1p if qc == 1 else w0p
                            tg = (lambda i: f"w1q1_{i}") if qc == 1 \
                                else (lambda i: f"w0s{i}")
                            sng = [
                                pool.tile(
                                    [128, 1024], F16,
                                    tag=tg(i), name=f"w1q{qc}_{i}"
                                )
                                for i in range(4)
                            ]
                            for i in range(4):
                                nc.sync.dma_start(
                                    sng[i][:, :],
                                    wih[s, qc][:, i * 1024:(i + 1) * 1024],
                                )
                            sweep_w[key] = sng
                        elif qc == 0:
                            sng = [
                                w0p.tile(
                                    [128, 1024], F16,
                                    tag=f"w0s{i}", name=f"w{s}s_{i}"
                                )
                                for i in range(4)
                            ]
                            for i in range(4):
                                nc.sync.dma_start(
                                    sng[i][:, :],
                                    wih[s, 0][:, i * 1024:(i + 1) * 1024],
                                )
                            sweep_w[key] = sng
                        else:
                            wq = wihp.tile(
                                [128, 4 * 1024], F16, tag="wih", name="wq"
                            )
                            nc.sync.dma_start(wq[:, :], wih[s, qc])
                            sweep_w[key] = wq
                    w = sweep_w[key]
                    if isinstance(w, list):
                        return w[kci][:, ml * 128:(ml + 1) * 128]
                    return w[:, kci * 1024 + ml * 128:
                             kci * 1024 + (ml + 1) * 128]

                def _rhs(qc, kci):
                    return xt_sbq[qc][:, kci * BL:(kci + 1) * BL]

                g0 = {}
                for wave in waves:
                    pst = {
                        ml: psump.tile([128, BL], F32, tag="ps",
                                       name=f"psA{ml}")
                        for ml in wave
                    }
                    for qc in range(NQ):
                        for kci in range(4):
                            kc = qc * 4 + kci
                            for ml in wave:
                                nc.tensor.matmul(
                                    pst[ml][:, :],
                                    _lhs(qc, kci, ml),
                                    _rhs(qc, kci),
                                    start=(kc == 0),
                                    stop=(kc == NKX - 1),
                                )
                    for ml in wave:
                        gi, j = ml // 2, ml % 2
                        q = (2 * s + j) * 4 + gi
                        # The xg save is the ONLY reader of the PSUM tile
                        # (the step-0 act below reads the SBUF copy), and
                        # saves alternate DVE/ACT so the 8 bank releases
                        # drain on two queues in parallel -- the next sweep
                        # needs the banks within ~1us of this one ending.
                        if ml % 2 == 0:
                            nc.vector.tensor_scalar_add(
                                xg_sb[:, q * BL:(q + 1) * BL],
                                pst[ml][:, :],
                                bb_sb[:, q:q + 1],
                            )
                        else:
                            nc.scalar.add(
                                xg_sb[:, q * BL:(q + 1) * BL],
                                pst[ml][:, :],
                                bb_sb[:, q:q + 1],
                            )
                        g = gatesp.tile(
                            [128, BL], F32, tag=f"g{gi}", name=f"gA{gi}{j}"
                        )
                        fn = AF.Tanh if gi == 2 else AF.Sigmoid
                        # step-0 gate: rb carries only 0.01*rowsum(W_hh);
                        # b_ih+b_hh is already inside the xg copy
                        nc.scalar.activation(
                            g[:, :], xg_sb[:, q * BL:(q + 1) * BL], fn,
                            bias=r_sb[:, q:q + 1],
                        )
                        g0[(gi, j)] = g
                for j in range(2):
                    hc = 2 * s + j
                    sl = slice(hc * BL, (hc + 1) * BL)
                    t0 = gatesp.tile([128, BL], F32, tag="t0")
                    th = gatesp.tile([128, BL], F32, tag="th")
                    nc.vector.tensor_mul(t0[:, :], g0[(0, j)][:, :], g0[(2, j)][:, :])
                    # c0 = f*INIT + i*g  (c_prev is the 0.01 constant)
                    nc.vector.scalar_tensor_tensor(
                        c0[:, sl], g0[(1, j)][:, :], INIT, t0[:, :],
                        ALU.mult, ALU.add,
                    )
                    nc.scalar.activation(th[:, :], c0[:, sl], AF.Tanh)
                    nc.vector.tensor_mul(h0[:, sl], g0[(3, j)][:, :], th[:, :])
            # W_hh streams in right after the wih tiles. All gi0/1 halves
            # first: step 1's pass 1 (i/f gates only) consumes them at the
            # rate they arrive, and the gi2/3 halves land exactly while
            # pass 1 runs -- a clean dovetail with the serial DMA unit.
            for h in range(2):
                for kc in range(NKH):
                    nc.sync.dma_start(
                        whh_sb[kc][h][:, :],
                        whh[kc][:, h * (G4 // 2):(h + 1) * (G4 // 2)],
                    )
            nc.sync.dma_start(hs[0], h0[:, :])

            # ---- Phase B: recurrent steps 1..14 ----
            def _cell(gt, hc, c_prev, c_new, h_new, pool=False):
                sl = slice(hc * BL, (hc + 1) * BL)
                t0 = gatesp.tile([128, BL], F32, tag="t0", name="t0")
                t1 = gatesp.tile([128, BL], F32, tag="t1", name="t1")
                th = gatesp.tile([128, BL], F32, tag="th", name="th")
                # during step-1 pass 2 the DVE is oversubscribed (~130%);
                # the idle Pool engine takes the two products there
                eng = nc.gpsimd if pool else nc.vector
                eng.tensor_mul(t0[:, :], gt[0][:, :], gt[2][:, :])
                eng.tensor_mul(t1[:, :], gt[1][:, :], c_prev[:, sl])
                nc.vector.tensor_add(c_new[:, sl], t0[:, :], t1[:, :])
                nc.scalar.activation(th[:, :], c_new[:, sl], AF.Tanh)
                nc.vector.tensor_mul(h_new[:, sl], gt[3][:, :], th[:, :])

            h_prev, c_prev = h0, c0

            # ---- Step 1: two passes dovetailing with the W_hh stream.
            # Pass 1 computes the i/f gates for every hc using only the
            # gi0/1 weight halves (which arrive first, at 2x chunk rate);
            # pass 2 computes g/o gates + cell updates while nothing is
            # left to wait for. Pass-1 gates park in the g01_sb strip.
            h_new = statep.tile([128, NKH * BL], F16, tag="h", name="h1")
            c_new = statep.tile([128, NKH * BL], F32, tag="c", name="c1")
            p1 = {}

            def _drain01(hc):
                for gi in (0, 1):
                    q = hc * 4 + gi
                    ps = p1[(hc, gi)]
                    nc.vector.tensor_add(
                        ps[:, :], ps[:, :], xg_sb[:, q * BL:(q + 1) * BL]
                    )
                    o = (hc * 2 + gi) * BL
                    nc.scalar.activation(
                        g01_sb[:, o:o + BL], ps[:, :], AF.Sigmoid
                    )

            for hc in range(NKH):
                for gi in (0, 1):
                    m = gi * NKH + hc
                    ps = psump.tile([128, BL], F32, tag="ps", name="ps1a")
                    kcs = (
                        list(range(NKH - 1)) if hc == 0
                        else list(range(NKH))
                    )
                    for kc in kcs:
                        nc.tensor.matmul(
                            ps[:, :],
                            whh_sb[kc][0][:, (m % 16) * 128:
                                          (m % 16) * 128 + 128],
                            h_prev[:, kc * BL:(kc + 1) * BL],
                            start=(kc == 0),
                            stop=(kc == NKH - 1),
                        )
                    p1[(hc, gi)] = ps
                if hc == 1:
                    for gi in (0, 1):
                        m = gi * NKH
                        nc.tensor.matmul(
                            p1[(0, gi)][:, :],
                            whh_sb[NKH - 1][0][:, (m % 16) * 128:
                                               (m % 16) * 128 + 128],
                            h_prev[:, (NKH - 1) * BL:NKH * BL],
                            start=False,
                            stop=True,
                        )
                    _drain01(0)
                    _drain01(1)
                elif hc >= 2:
                    _drain01(hc)
            for hc in range(NKH):
                gt23 = []
                for gi in (2, 3):
                    m = gi * NKH + hc
                    ps = psump.tile([128, BL], F32, tag="ps", name="ps1b")
                    for kc in range(NKH):
                        nc.tensor.matmul(
                            ps[:, :],
                            whh_sb[kc][1][:, (m % 16) * 128:
                                          (m % 16) * 128 + 128],
                            h_prev[:, kc * BL:(kc + 1) * BL],
                            start=(kc == 0),
                            stop=(kc == NKH - 1),
                        )
                    nc.vector.tensor_add(
                        ps[:, :], ps[:, :],
                        xg_sb[:, (hc * 4 + gi) * BL:(hc * 4 + gi + 1) * BL],
                    )
                    g = gatesp.tile([128, BL], F32, tag=f"g{gi}", name=f"g1b{gi}")
                    fn = AF.Tanh if gi == 2 else AF.Sigmoid
                    nc.scalar.activation(g[:, :], ps[:, :], fn)
                    gt23.append(g)
                o = hc * 2 * BL
                gt = [
                    g01_sb[:, o:o + BL], g01_sb[:, o + BL:o + 2 * BL],
                    gt23[0], gt23[1],
                ]
                # Pool for throughput on early chunks; DVE for the last
                # two, whose cell latency gates step-2's deferred matmuls
                _cell(gt, hc, c_prev, c_new, h_new, pool=(hc < 6))
            nc.sync.dma_start(hs[1], h_new[:, :])
            h_prev, c_prev = h_new, c_new

            for t in range(2, T):
                h_new = statep.tile([128, NKH * BL], F16, tag="h")
                c_new = statep.tile([128, NKH * BL], F32, tag="c")
                for hc in range(NKH):
                    pss = []
                    for gi in range(4):
                        m = gi * NKH + hc
                        ps = psump.tile([128, BL], F32, tag="ps", name="psB")
                        # For the first hc group of a step, defer the kc=6
                        # and kc=7 chunks: the previous step's last h chunks
                        # land ~1-3us after its last matmul, so give the PE
                        # runway before consuming them.
                        kcs = (
                            list(range(NKH - 3)) if hc == 0
                            else list(range(NKH))
                        )
                        mh, mo = m // 16, (m % 16) * 128
                        for kc in kcs:
                            nc.tensor.matmul(
                                ps[:, :],
                                whh_sb[kc][mh][:, mo:mo + 128],
                                h_prev[:, kc * BL:(kc + 1) * BL],
                                start=(kc == 0),
                                stop=(kc == NKH - 1),
                            )
                        pss.append(ps)
                    if hc == 0:
                        for kc in (NKH - 3, NKH - 2, NKH - 1):
                            for gi in range(4):
                                m = gi * NKH
                                nc.tensor.matmul(
                                    pss[gi][:, :],
                                    whh_sb[kc][m // 16][:, (m % 16) * 128:
                                                        (m % 16) * 128 + 128],
                                    h_prev[:, kc * BL:(kc + 1) * BL],
                                    start=False,
                                    stop=(kc == NKH - 1),
                                )
                    gt = []
                    for gi in range(4):
                        q = hc * 4 + gi
                        ps = pss[gi]
                        nc.vector.tensor_add(
                            ps[:, :], ps[:, :], xg_sb[:, q * BL:(q + 1) * BL]
                        )
                        g = gatesp.tile([128, BL], F32, tag=f"g{gi}", name=f"gB{gi}")
                        fn = AF.Tanh if gi == 2 else AF.Sigmoid
                        nc.scalar.activation(g[:, :], ps[:, :], fn)
                        gt.append(g)
                    _cell(gt, hc, c_prev, c_new, h_new)
                    if t == T - 1 and hc == 3:
                        # last step: flush hs progressively so the
                        # end-of-kernel drain only waits on a quarter tile
                        nc.sync.dma_start(
                            hs[t][:, :4 * BL], h_new[:, :4 * BL]
                        )
                    if t == T - 1 and hc == 5:
                        nc.sync.dma_start(
                            hs[t][:, 4 * BL:6 * BL], h_new[:, 4 * BL:6 * BL]
                        )
                if t == T - 1:
                    nc.sync.dma_start(hs[t][:, 6 * BL:], h_new[:, 6 * BL:])
                else:
                    nc.sync.dma_start(hs[t], h_new[:, :])
                h_prev, c_prev = h_new, c_new

    nc.compile()
    return nc


def timeline_ns():
    from concourse.timeline_sim import TimelineSim
    nc = _get_nc()
    ts = TimelineSim(nc)
    ts.simulate()
    return ts.time


def _get_nc():
    global _cached_nc
    if _cached_nc is None:
        _cached_nc = _build()
    return _cached_nc


def _pack_weights(W_ih, W_hh, b_ih, b_hh):
    f16 = np.float16
    wt = W_ih.T.astype(np.float32)                          # [IN, 4H]
    # [kc, p, gi, s, j, col] -> [s, kc, p, gi, j, col]
    wperm = wt.reshape(NKX, 128, 4, NSW, 2, 128).transpose(3, 0, 1, 2, 4, 5)
    # regroup kc into quads: [s, qc, kci, p, gi, j, col] -> [s, qc, p, kci, ...]
    wperm = wperm.reshape(NSW, NQ, 4, 128, 1024).transpose(0, 1, 3, 2, 4)
    wih_host = np.ascontiguousarray(
        wperm.reshape(NSW, NQ, 128, 4 * 1024)
    ).astype(f16)
    whh_host = np.ascontiguousarray(W_hh.T.reshape(NKH, 128, G4)).astype(f16)
    bias = (b_ih + b_hh).astype(np.float32)                 # [4096]
    r = (INIT * W_hh.sum(axis=1)).astype(np.float32)

    def _qlay(v):  # [4096] in (gi, hc, p) order -> [128, q=hc*4+gi]
        return np.ascontiguousarray(
            v.reshape(4, NKH, 128).transpose(1, 0, 2).reshape(NM, 128).T
        )

    return wih_host, whh_host, _qlay(r), _qlay(bias)


def kernel(x, W_ih, W_hh, b_ih, b_hh):
    global LAST_EXEC_NS, LAST_RESULTS
    nc = _get_nc()
    x = np.asarray(x, np.float32)
    W_ih = np.asarray(W_ih, np.float32)
    W_hh = np.asarray(W_hh, np.float32)
    b_ih = np.asarray(b_ih, np.float32)
    b_hh = np.asarray(b_hh, np.float32)

    wih_host, whh_host, r_host, b_host = _pack_weights(W_ih, W_hh, b_ih, b_hh)

    in_maps = []
    for c in range(NCORES):
        xa = np.ascontiguousarray(x[c * BL:(c + 1) * BL].T)
        xt_host = xa.astype(np.float16).reshape(NKX, 128, BL)
        in_maps.append({
            "wih": wih_host, "whh": whh_host, "xt": xt_host,
            "rb": r_host, "bb": b_host,
        })

    trace = os.environ.get("LSTM_TRACE") == "1"
    res = run_bass_kernel_spmd(
        nc, in_maps, core_ids=list(range(NCORES)), trace=trace
    )
    LAST_EXEC_NS = res.exec_time_ns
    LAST_RESULTS = res

    out = np.empty((T, B, H), np.float32)
    for c in range(NCORES):
        a = res.results[c]["hs"].astype(np.float32).reshape(T, 128, NKH, BL)
        out[:, c * BL:(c + 1) * BL, :] = (
            a.transpose(0, 3, 2, 1).reshape(T, BL, H)
        )
    return out
